# revision 55
# baseline (speedup 1.0000x reference)
"""Trainium2 kernel for cosine-similarity attention (nn_Attention_30202210025712).

reference math (num_of_method==2 path, the only implemented one):
    qn = q / ||q||_row ; kn = k / ||k||_row
    scores = hyper_c * qn @ kn.T          # [N, M]
    p_attn = softmax(scores, axis=-1)     # [N, M]  <-- 256MB output, memory-bound
    res    = p_attn @ v                   # [N, D]
    return (res, p_attn)

Strategy: shard query rows across 8 NeuronCores (data parallel, K/V replicated).
Host prep (O(N*D), negligible): fold hyper_c into normalized q, cast to bf16,
and lay out q^T / k^T / [v|1] so the device kernel needs no transposes.

Per core, scores are computed TRANSPOSED (S_T[m, n] = sum_d k^[m,d] q^[n,d]) so
that (a) the P@V contraction over m runs on the TensorEngine with m on
partitions, and (b) the softmax row-sum over m falls out of the same matmul via
a ones-column appended to V.  exp runs on ScalarE straight out of PSUM.  The
normalized p tiles stream out as fully-contiguous ~2MB DMA writes alternating
between the SP-HWDGE and Pool-SWDGE queues (the memory roofline term:
32MB/core).  Host re-transposes the per-core [m, n] output into [n, m].
"""

import sys

for _p in ("/opt/trn_rl_repo",):
    if _p not in sys.path:
        sys.path.insert(0, _p)

import numpy as np
import ml_dtypes

import concourse.bass as bass
import concourse.mybir as mybir
import concourse.tile as tile
from concourse.bass import ts, ds
from concourse.bass_utils import run_bass_kernel_spmd

BF16 = ml_dtypes.bfloat16
N_CORES = 8


def _split_multi_wait_instructions(nc):
    """Walrus on this toolchain encodes at most ONE sync wait per instruction
    ('Too many sync wait commands' otherwise).  Tile freely emits several.
    Post-pass: for any instruction with N>1 waits, hoist N-1 of them onto
    fresh same-engine drain instructions inserted immediately before it —
    the engine blocks on each in program order, so semantics are identical.

    EXCEPTION: the PE queue is a partial-reorder window that pulls LDWEIGHTS
    ahead of in-flight work, so a PE drain carrying a wait does NOT reliably
    fence the next matmul's weight fetch (observed as cold-run corruption).
    Multi-wait PE instructions instead route ALL their waits through an
    SP-side drain chain that bumps a dedicated gate semaphore; the PE
    instruction carries the single gate wait.  Gates are cleared at the
    kernel tail so repeat executions see them at zero."""
    gate_pool = list(getattr(nc, "_pe_gate_sems", []))
    pe_gates = []

    def _mk_drain(engine, on_wait, on_update=()):
        d = mybir.InstDrain(
            name=nc.get_next_instruction_name(),
            ins=[], outs=[], bass_is_fusable=False,
        )
        d.engine = engine
        d.sync_info = mybir.SyncInfo(on_wait=list(on_wait),
                                     on_update=list(on_update))
        nc.register_instruction(d)
        return d

    for func in nc.m.functions:
        for bb in func.blocks:
            insts = list(bb.instructions)
            if not any(
                getattr(i, "sync_info", None) is not None
                and i.sync_info.on_wait and len(i.sync_info.on_wait) > 1
                for i in insts
            ):
                continue
            new_list = []
            for inst in insts:
                si = getattr(inst, "sync_info", None)
                if si is not None and si.on_wait and len(si.on_wait) > 1:
                    waits = list(si.on_wait)
                    if inst.engine == mybir.EngineType.PE:
                        assert gate_pool, "ran out of pre-allocated PE gate sems"
                        gate = gate_pool.pop()
                        pe_gates.append(gate)
                        for j, w in enumerate(waits):
                            upd = []
                            if j == len(waits) - 1:
                                upd = [mybir.SyncUpdate(
                                    sync_type="semaphore", id=gate.num,
                                    ant_name=gate.name,
                                    update_mode="sem-inc", update_value=1,
                                    update_reg=None)]
                            new_list.append(_mk_drain(
                                mybir.EngineType.SP, [w], upd))
                        gate_wait = mybir.SyncWait(
                            sync_type="semaphore", id=gate.num,
                            ant_name=gate.name, wait_mode="sem-ge-imm",
                            wait_value=1, wait_reg=None)
                        inst.sync_info = mybir.SyncInfo(
                            on_wait=[gate_wait],
                            on_update=list(si.on_update or []))
                    else:
                        for w in waits[:-1]:
                            new_list.append(_mk_drain(inst.engine, [w]))
                        inst.sync_info = mybir.SyncInfo(
                            on_wait=[waits[-1]],
                            on_update=list(si.on_update or []))
                new_list.append(inst)
            bb.instructions = new_list
    if pe_gates:
        # clear the gates at the very tail (current bb is after the final
        # all-engine barrier) so a re-execution of the NEFF starts at zero
        nc.gpsimd.sem_clear(range(min(g.num for g in pe_gates),
                                  max(g.num for g in pe_gates) + 1))


# Full-problem dimensions (hardcoded per spec: q/k/v are [8192, 64] f32).
FULL_N = 8192
FULL_M = 8192
FULL_D = 64


def _default_groups(Ncore):
    """Group widths (query rows per softmax group).  A small first group gets
    the output DMA stream started early; a small last group trims the tail.
    Widths must be in {64, 128, 256} (PSUM bank alignment)."""
    if Ncore == FULL_N // N_CORES:
        return [64, 128, 256, 256, 256, 64]
    g = (Ncore + 255) // 256
    base = Ncore // g
    rem = Ncore - base * g
    return [base + (1 if i < rem else 0) for i in range(g)]


class Cfg:
    def __init__(self, M=FULL_M, Ncore=FULL_N // N_CORES, D=FULL_D,
                 GROUPS=None, B_EXP=4, B_DMA=16, STAGE_BUFS=4, S_BUFS=2,
                 E_BUFS=2, DMA_ENGS=("sync", "gpsimd"), SPLIT_LOADS=True,
                 BF16_STAGE=True):
        assert M % 256 == 0 and D == 64
        self.M = M              # number of keys (replicated on every core)
        self.Ncore = Ncore      # query rows per core
        self.D = D
        self.GROUPS = list(GROUPS) if GROUPS else _default_groups(Ncore)
        assert sum(self.GROUPS) == Ncore and max(self.GROUPS) <= 256
        self.G = len(self.GROUPS)
        self.MT = M // 128      # 128-key m-tiles
        self.B_EXP = B_EXP      # m-tiles per exp() block at group width 256
        self.B_DMA = B_DMA      # m-tiles per output DMA batch at width 256
        # Per-group block sizes: keep exp width (B_EXP*256 elems) and DMA
        # chunk bytes (B_DMA*256 words/partition) constant across groups, so
        # narrow groups use proportionally more m-tiles per block.
        self.B_EXP_G = [min(self.MT, B_EXP * 256 // n) for n in self.GROUPS]
        self.B_DMA_G = [min(self.MT, B_DMA * 256 // n) for n in self.GROUPS]
        for n, be, bd in zip(self.GROUPS, self.B_EXP_G, self.B_DMA_G):
            assert self.MT % be == 0 and self.MT % bd == 0, (n, be, bd)
        self.STAGE_BUFS = STAGE_BUFS
        self.S_BUFS = S_BUFS
        self.E_BUFS = E_BUFS
        self.DMA_ENGS = DMA_ENGS
        self.SPLIT_LOADS = SPLIT_LOADS
        self.BF16_STAGE = BF16_STAGE

    def p_chunks(self):
        """Emission-order (group, batch) output chunks of the flat p_out:
        yields (g, b, row0, n_g, b_dma, offset_words, size_words)."""
        ofs = 0
        row0 = 0
        for g, n_g in enumerate(self.GROUPS):
            bd = self.B_DMA_G[g]
            for b in range(self.MT // bd):
                size = 128 * bd * n_g
                yield g, b, row0, n_g, bd, ofs, size
                ofs += size
            row0 += n_g


def build_nc(cfg: Cfg):
    """Build the single-core Bass graph (same graph runs SPMD on all cores)."""
    fp32 = mybir.dt.float32
    bf16 = mybir.dt.bfloat16
    M, MT, D = cfg.M, cfg.MT, cfg.D

    nc = bass.Bass()
    # Pre-allocate gate semaphores for the multi-wait splitter (see
    # _split_multi_wait_instructions) so their IDs are never shared with
    # Tile's own semaphores.
    nc._pe_gate_sems = [nc.alloc_semaphore(f"pegate{i}") for i in range(24)]
    # inputs (host pre-laid-out, bf16).  NOTE: all matmuls keep their
    # contraction operands at base partition 0 — packing two K=64 matmuls
    # into row-groups 0/64 of the PE array crashes the device when both
    # write the same PSUM tile, and is racy even into separate tiles.
    qT = nc.declare_dram_parameter("qT", [64, cfg.Ncore], bf16, isOutput=False)
    kT = nc.declare_dram_parameter("kT", [64, M], bf16, isOutput=False)
    vE = nc.declare_dram_parameter("vE", [128, MT * 65], bf16, isOutput=False)
    # outputs.  p_out is a flat buffer of contiguous per-(group, batch)
    # chunks; chunk [p, i, c] maps to p_attn[row0 + c, (b*B_DMA + i)*128 + p].
    # Flat layout keeps every output DMA one fully-contiguous ~2MB write.
    p_out = nc.declare_dram_parameter(
        "p_out", [cfg.Ncore * M], fp32, isOutput=True)
    resT_out = nc.declare_dram_parameter("resT_out", [D, cfg.Ncore], fp32, isOutput=True)

    chunks = list(cfg.p_chunks())
    chunk_at = {(g, b): (row0, n_g, bd, ofs, size)
                for g, b, row0, n_g, bd, ofs, size in chunks}

    with tile.TileContext(nc) as tc:
        with (
            tc.tile_pool(name="const", bufs=1) as cpool,
            tc.tile_pool(name="E", bufs=cfg.E_BUFS) as epool,
            tc.tile_pool(name="stage", bufs=cfg.STAGE_BUFS) as spool,
            tc.tile_pool(name="stage_bf", bufs=cfg.STAGE_BUFS) as sbfpool,
            tc.tile_pool(name="rsmall", bufs=2) as rpool,
            tc.tile_pool(name="fence", bufs=max(2, cfg.G)) as fpool,
            tc.tile_pool(name="S", bufs=cfg.S_BUFS, space="PSUM") as s_psum,
            tc.tile_pool(name="ACC", bufs=2, space="PSUM") as acc_psum,
            tc.tile_pool(name="RB", bufs=1, space="PSUM") as r_psum,
        ):
            # resident inputs
            qT_sb = cpool.tile([64, cfg.Ncore], bf16)
            kT_sb = cpool.tile([64, M], bf16)
            vE_sb = cpool.tile([128, MT * 65], bf16)
            ones_sb = cpool.tile([1, 128], fp32)
            resT_sb = cpool.tile([D, cfg.Ncore], fp32)
            if cfg.SPLIT_LOADS:
                # Split the resident-input loads into chunks so group 0's
                # first matmuls start as soon as their k/q/v slices land
                # instead of waiting for whole-tensor DMAs.
                n0 = cfg.GROUPS[0]
                nc.sync.dma_start(qT_sb[:, 0:n0], qT[:, 0:n0])
                for c4 in range(4):
                    sl = ds(c4 * M // 4, M // 4)
                    nc.sync.dma_start(kT_sb[:, sl], kT[:, sl])
                    sl2 = ds(c4 * (MT // 4) * 65, (MT // 4) * 65)
                    nc.sync.dma_start(vE_sb[:, sl2], vE[:, sl2])
                if n0 < cfg.Ncore:
                    nc.sync.dma_start(qT_sb[:, n0:], qT[:, n0:])
            else:
                nc.sync.dma_start(qT_sb[:], qT[:])
                nc.sync.dma_start(kT_sb[:], kT[:])
                nc.sync.dma_start(vE_sb[:], vE[:])
            # DVE memset (not gpsimd): the R-broadcast matmul depends on both
            # this and the DVE reciprocal; PE instructions fit only ONE sync
            # wait, so both deps must land on the same (DVE) semaphore.
            nc.vector.memset(ones_sb[:], 1.0)
            # Dummy exp up front: forces the ACT spline-table load to happen
            # during the input-DMA window.  On a freshly loaded NEFF the first
            # real exp otherwise races the table load and yields garbage.
            scratch_act = cpool.tile([1, 2], fp32)
            nc.scalar.activation(scratch_act[:], ones_sb[0:1, 0:2],
                                 mybir.ActivationFunctionType.Exp)

            row0 = 0
            dma_idx = 0
            for g, n in enumerate(cfg.GROUPS):
                b_exp = cfg.B_EXP_G[g]
                E = epool.tile([128, MT * n], bf16, tag="E")  # exp(S_T)
                acc = acc_psum.tile([65, n], fp32, tag="acc")  # [V|1]^T @ E
                for blk in range(MT // b_exp):
                    S = s_psum.tile([128, b_exp * n], fp32, tag="S")
                    for i in range(b_exp):
                        t = blk * b_exp + i
                        nc.tensor.matmul(
                            S[:, ts(i, n)], kT_sb[:, ts(t, 128)],
                            qT_sb[:, ds(row0, n)],
                            start=True, stop=True,
                        )
                    nc.scalar.activation(
                        E[:, ds(blk * b_exp * n, b_exp * n)], S[:],
                        mybir.ActivationFunctionType.Exp,
                    )
                    for i in range(b_exp):
                        t = blk * b_exp + i
                        nc.tensor.matmul(
                            acc[:], vE_sb[:, ts(t, 65)], E[:, ts(t, n)],
                            start=(t == 0), stop=(t == MT - 1),
                        )
                # softmax denominators for this group: acc row 64 holds sums
                r = rpool.tile([1, n], fp32, tag="r")
                nc.vector.reciprocal(r[:], acc[64:65, :])
                Rps = r_psum.tile([128, n], fp32, tag="Rps")
                nc.tensor.matmul(Rps[:], ones_sb[:], r[:], start=True, stop=True)
                Rbf = rpool.tile([128, n], bf16, tag="Rbf")
                nc.vector.tensor_copy(Rbf[:], Rps[:])
                Rf32 = rpool.tile([128, n], fp32, tag="Rf32")
                nc.vector.tensor_copy(Rf32[:], Rps[:])
                # res_T slice for this group = acc[0:64] * r (broadcast over d)
                nc.vector.tensor_tensor(
                    resT_sb[:, ds(row0, n)], acc[0:D, :], Rf32[0:D, :],
                    mybir.AluOpType.mult,
                )
                # ACT fence on DVE: one op that waits for the group's last exp,
                # so the phase-C TTs' E-read deps are elided and each TT keeps
                # a single sync wait (its stage-slot WAR vs the output DMA) —
                # ISA instructions fit only one wait.
                fence = fpool.tile([1, 1], bf16, tag="fence")
                nc.vector.tensor_copy(fence[:], E[0:1, MT * n - 1:MT * n])
                # normalize + stream p tiles out.  Alternate DMA issue between
                # SP-HWDGE and Pool-SWDGE — independent descriptor queues, so
                # the output stream pipelines across both.  SWDGE batches use
                # a bf16 stage (DVE runs the multiply at 2x for 2-byte in/out)
                # and cast bf16->f32 during the DMA; HWDGE can't cast, so its
                # batches stage in f32.
                b_dma = cfg.B_DMA_G[g]
                for b in range(MT // b_dma):
                    _, n_g, bd, ofs, size = chunk_at[(g, b)]
                    use_swdge = (dma_idx % 2 == 1) and "gpsimd" in cfg.DMA_ENGS
                    dma_idx += 1
                    if use_swdge and cfg.BF16_STAGE:
                        dst = sbfpool.tile([128, bd * n], bf16, tag="stage_bf")
                        eng = nc.gpsimd
                    else:
                        dst = spool.tile([128, bd * n], fp32, tag="stage")
                        eng = nc.gpsimd if use_swdge else nc.sync
                    nc.vector.tensor_tensor(
                        dst[:].rearrange("p (i c) -> p i c", i=bd),
                        E[:, ds(b * bd * n, bd * n)].rearrange(
                            "p (i c) -> p i c", i=bd),
                        Rbf[:, None, :].to_broadcast([128, bd, n]),
                        mybir.AluOpType.mult,
                    )
                    eng.dma_start(
                        p_out[ofs:ofs + size].rearrange("(p x) -> p x", p=128),
                        dst[:],
                    )
                row0 += n
            # res goes out on the (otherwise idle) ACT HWDGE queue so it does
            # not delay the last p batches on the SP queue.
            nc.scalar.dma_start(resT_out[:], resT_sb[:])
    _split_multi_wait_instructions(nc)
    return nc


def prep_inputs(query, key, value, hyper_c, cfg: Cfg, n_cores=N_CORES):
    """Host-side layout prep: normalize rows, fold hyper_c, bf16, transpose."""
    q = np.asarray(query, dtype=np.float32)
    k = np.asarray(key, dtype=np.float32)
    v = np.asarray(value, dtype=np.float32)
    hc = float(hyper_c)
    MT, D = cfg.MT, cfg.D

    qn_norm = np.linalg.norm(q, axis=1, keepdims=True)
    kn_norm = np.linalg.norm(k, axis=1, keepdims=True)
    qn = (q * (hc / np.where(qn_norm == 0, 1.0, qn_norm))).astype(BF16)
    kn = (k / np.where(kn_norm == 0, 1.0, kn_norm)).astype(BF16)

    # kT: [64, M] = normalized keys transposed (d on partitions)
    kT_h = np.ascontiguousarray(kn.T)
    # vE: [128, MT*65]; row p, col t*65+c -> v[t*128+p, c] for c<64, 1.0 at c=64
    vE = np.ones((MT, 128, 65), dtype=np.float32)
    vE[:, :, :D] = v.reshape(MT, 128, D)
    vE_h = np.ascontiguousarray(vE.astype(BF16).transpose(1, 0, 2).reshape(128, MT * 65))

    qT_all = qn.T  # [64, N]
    in_maps = []
    for c in range(n_cores):
        qT_h = np.ascontiguousarray(qT_all[:, c * cfg.Ncore:(c + 1) * cfg.Ncore])
        in_maps.append({"qT": qT_h, "kT": kT_h, "vE": vE_h})
    return in_maps


def gather_outputs(results, cfg: Cfg, n_cores=N_CORES):
    N = cfg.Ncore * n_cores
    p_attn = np.empty((N, cfg.M), dtype=np.float32)
    res = np.empty((N, cfg.D), dtype=np.float32)
    chunks = list(cfg.p_chunks())
    for c in range(n_cores):
        po = results[c]["p_out"]          # flat [Ncore*M]
        rt = results[c]["resT_out"]       # [D, Ncore]
        base = c * cfg.Ncore
        for g, b, row0, n_g, bd, ofs, size in chunks:
            chunk = po[ofs:ofs + size].reshape(128, bd, n_g)
            cols = b * bd * 128
            # chunk[p, i, c2] -> p_attn[base+row0+c2, cols + i*128 + p]
            p_attn[base + row0:base + row0 + n_g,
                   cols:cols + bd * 128] = (
                chunk.transpose(2, 1, 0).reshape(n_g, bd * 128))
        res[base:base + cfg.Ncore, :] = rt.T
    return res, p_attn


_NC_CACHE = {}


def _get_nc(cfg_key=None):
    if cfg_key is None:
        cfg = Cfg()
        key = "full"
    else:
        cfg, key = cfg_key
    if key not in _NC_CACHE:
        _NC_CACHE[key] = (build_nc(cfg), cfg)
    return _NC_CACHE[key]


def run(inputs, trace=False, cfg=None, warmup=True, **spmd_kwargs):
    """Full pipeline; returns ((res, p_attn), BassKernelResults).

    warmup: the FIRST execution of a freshly loaded NEFF can race the cold
    DMA/engine pipelines (observed: corrupted tails of the first score
    matmuls on cold runs only; every warm execution is deterministic and
    correct).  Run once to warm the pipelines, then return the second run.
    """
    if cfg is None:
        nc, cfg = _get_nc()
    else:
        nc = build_nc(cfg)
    in_maps = prep_inputs(inputs["query"], inputs["key"], inputs["value"],
                          inputs.get("hyper_c", 1), cfg)
    core_ids = list(range(N_CORES))
    if warmup:
        run_bass_kernel_spmd(nc, in_maps, core_ids=core_ids, trace=False)
    out = run_bass_kernel_spmd(nc, in_maps, core_ids=core_ids,
                               trace=trace, **spmd_kwargs)
    return gather_outputs(out.results, cfg), out


def kernel(**inputs):
    (res, p_attn), _ = run(inputs, trace=False)
    return res, p_attn


# revision 56
# speedup vs baseline: 1.0260x; 1.0260x over previous
"""Trainium2 kernel for cosine-similarity attention (nn_Attention_30202210025712).

reference math (num_of_method==2 path, the only implemented one):
    qn = q / ||q||_row ; kn = k / ||k||_row
    scores = hyper_c * qn @ kn.T          # [N, M]
    p_attn = softmax(scores, axis=-1)     # [N, M]  <-- 256MB output, memory-bound
    res    = p_attn @ v                   # [N, D]
    return (res, p_attn)

Strategy: shard query rows across 8 NeuronCores (data parallel, K/V replicated).
Host prep (O(N*D), negligible): fold hyper_c into normalized q, cast to bf16,
and lay out q^T / k^T / [v|1] so the device kernel needs no transposes.

Per core, scores are computed TRANSPOSED (S_T[m, n] = sum_d k^[m,d] q^[n,d]) so
that (a) the P@V contraction over m runs on the TensorEngine with m on
partitions, and (b) the softmax row-sum over m falls out of the same matmul via
a ones-column appended to V.  exp runs on ScalarE straight out of PSUM.  The
normalized p tiles stream out as fully-contiguous ~2MB DMA writes alternating
between the SP-HWDGE and Pool-SWDGE queues (the memory roofline term:
32MB/core).  Host re-transposes the per-core [m, n] output into [n, m].
"""

import sys

for _p in ("/opt/trn_rl_repo",):
    if _p not in sys.path:
        sys.path.insert(0, _p)

import numpy as np
import ml_dtypes

import concourse.bass as bass
import concourse.mybir as mybir
import concourse.tile as tile
from concourse.bass import ts, ds
from concourse.bass_utils import run_bass_kernel_spmd

BF16 = ml_dtypes.bfloat16
N_CORES = 8


def _split_multi_wait_instructions(nc):
    """Walrus on this toolchain encodes at most ONE sync wait per instruction
    ('Too many sync wait commands' otherwise).  Tile freely emits several.
    Post-pass: for any instruction with N>1 waits, hoist N-1 of them onto
    fresh same-engine drain instructions inserted immediately before it —
    the engine blocks on each in program order, so semantics are identical.

    EXCEPTION: the PE queue is a partial-reorder window that pulls LDWEIGHTS
    ahead of in-flight work, so a PE drain carrying a wait does NOT reliably
    fence the next matmul's weight fetch (observed as cold-run corruption).
    Multi-wait PE instructions instead route ALL their waits through an
    SP-side drain chain that bumps a dedicated gate semaphore; the PE
    instruction carries the single gate wait.  Gates are cleared at the
    kernel tail so repeat executions see them at zero."""
    gate_pool = list(getattr(nc, "_pe_gate_sems", []))
    pe_gates = []

    def _mk_drain(engine, on_wait, on_update=()):
        d = mybir.InstDrain(
            name=nc.get_next_instruction_name(),
            ins=[], outs=[], bass_is_fusable=False,
        )
        d.engine = engine
        d.sync_info = mybir.SyncInfo(on_wait=list(on_wait),
                                     on_update=list(on_update))
        nc.register_instruction(d)
        return d

    for func in nc.m.functions:
        for bb in func.blocks:
            insts = list(bb.instructions)
            if not any(
                getattr(i, "sync_info", None) is not None
                and i.sync_info.on_wait and len(i.sync_info.on_wait) > 1
                for i in insts
            ):
                continue
            new_list = []
            for inst in insts:
                si = getattr(inst, "sync_info", None)
                if si is not None and si.on_wait and len(si.on_wait) > 1:
                    waits = list(si.on_wait)
                    if inst.engine == mybir.EngineType.PE:
                        assert gate_pool, "ran out of pre-allocated PE gate sems"
                        gate = gate_pool.pop()
                        pe_gates.append(gate)
                        for j, w in enumerate(waits):
                            upd = []
                            if j == len(waits) - 1:
                                upd = [mybir.SyncUpdate(
                                    sync_type="semaphore", id=gate.num,
                                    ant_name=gate.name,
                                    update_mode="sem-inc", update_value=1,
                                    update_reg=None)]
                            new_list.append(_mk_drain(
                                mybir.EngineType.SP, [w], upd))
                        gate_wait = mybir.SyncWait(
                            sync_type="semaphore", id=gate.num,
                            ant_name=gate.name, wait_mode="sem-ge-imm",
                            wait_value=1, wait_reg=None)
                        inst.sync_info = mybir.SyncInfo(
                            on_wait=[gate_wait],
                            on_update=list(si.on_update or []))
                    else:
                        for w in waits[:-1]:
                            new_list.append(_mk_drain(inst.engine, [w]))
                        inst.sync_info = mybir.SyncInfo(
                            on_wait=[waits[-1]],
                            on_update=list(si.on_update or []))
                new_list.append(inst)
            bb.instructions = new_list
    if pe_gates:
        # clear the gates at the very tail (current bb is after the final
        # all-engine barrier) so a re-execution of the NEFF starts at zero
        nc.gpsimd.sem_clear(range(min(g.num for g in pe_gates),
                                  max(g.num for g in pe_gates) + 1))


# Full-problem dimensions (hardcoded per spec: q/k/v are [8192, 64] f32).
FULL_N = 8192
FULL_M = 8192
FULL_D = 64


def _default_groups(Ncore):
    """Group widths (query rows per softmax group).  A small first group gets
    the output DMA stream started early; a small last group trims the tail.
    Widths must be in {64, 128, 256} (PSUM bank alignment)."""
    if Ncore == FULL_N // N_CORES:
        return [128, 256, 256, 256, 128]
    g = (Ncore + 255) // 256
    base = Ncore // g
    rem = Ncore - base * g
    return [base + (1 if i < rem else 0) for i in range(g)]


class Cfg:
    def __init__(self, M=FULL_M, Ncore=FULL_N // N_CORES, D=FULL_D,
                 GROUPS=None, B_EXP=4, B_DMA=16, STAGE_BUFS=4, S_BUFS=2,
                 E_BUFS=2, DMA_ENGS=("sync", "gpsimd"), SPLIT_LOADS=True,
                 BF16_STAGE=True):
        assert M % 256 == 0 and D == 64
        self.M = M              # number of keys (replicated on every core)
        self.Ncore = Ncore      # query rows per core
        self.D = D
        self.GROUPS = list(GROUPS) if GROUPS else _default_groups(Ncore)
        assert sum(self.GROUPS) == Ncore and max(self.GROUPS) <= 256
        self.G = len(self.GROUPS)
        self.MT = M // 128      # 128-key m-tiles
        self.B_EXP = B_EXP      # m-tiles per exp() block at group width 256
        self.B_DMA = B_DMA      # m-tiles per output DMA batch at width 256
        # Per-group block sizes: keep exp width (B_EXP*256 elems) and DMA
        # chunk bytes (B_DMA*256 words/partition) constant across groups, so
        # narrow groups use proportionally more m-tiles per block.
        self.B_EXP_G = [min(self.MT, B_EXP * 256 // n) for n in self.GROUPS]
        self.B_DMA_G = [min(self.MT, B_DMA * 256 // n) for n in self.GROUPS]
        for n, be, bd in zip(self.GROUPS, self.B_EXP_G, self.B_DMA_G):
            assert self.MT % be == 0 and self.MT % bd == 0, (n, be, bd)
        self.STAGE_BUFS = STAGE_BUFS
        self.S_BUFS = S_BUFS
        self.E_BUFS = E_BUFS
        self.DMA_ENGS = DMA_ENGS
        self.SPLIT_LOADS = SPLIT_LOADS
        self.BF16_STAGE = BF16_STAGE

    def p_chunks(self):
        """Emission-order (group, batch) output chunks of the flat p_out:
        yields (g, b, row0, n_g, b_dma, offset_words, size_words)."""
        ofs = 0
        row0 = 0
        for g, n_g in enumerate(self.GROUPS):
            bd = self.B_DMA_G[g]
            for b in range(self.MT // bd):
                size = 128 * bd * n_g
                yield g, b, row0, n_g, bd, ofs, size
                ofs += size
            row0 += n_g


def build_nc(cfg: Cfg):
    """Build the single-core Bass graph (same graph runs SPMD on all cores)."""
    fp32 = mybir.dt.float32
    bf16 = mybir.dt.bfloat16
    M, MT, D = cfg.M, cfg.MT, cfg.D

    nc = bass.Bass()
    # Pre-allocate gate semaphores for the multi-wait splitter (see
    # _split_multi_wait_instructions) so their IDs are never shared with
    # Tile's own semaphores.
    nc._pe_gate_sems = [nc.alloc_semaphore(f"pegate{i}") for i in range(24)]
    # inputs (host pre-laid-out, bf16).  NOTE: all matmuls keep their
    # contraction operands at base partition 0 — packing two K=64 matmuls
    # into row-groups 0/64 of the PE array crashes the device when both
    # write the same PSUM tile, and is racy even into separate tiles.
    qT = nc.declare_dram_parameter("qT", [64, cfg.Ncore], bf16, isOutput=False)
    kT = nc.declare_dram_parameter("kT", [64, M], bf16, isOutput=False)
    vE = nc.declare_dram_parameter("vE", [128, MT * 65], bf16, isOutput=False)
    # outputs.  p_out is a flat buffer of contiguous per-(group, batch)
    # chunks; chunk [p, i, c] maps to p_attn[row0 + c, (b*B_DMA + i)*128 + p].
    # Flat layout keeps every output DMA one fully-contiguous ~2MB write.
    p_out = nc.declare_dram_parameter(
        "p_out", [cfg.Ncore * M], fp32, isOutput=True)
    resT_out = nc.declare_dram_parameter("resT_out", [D, cfg.Ncore], fp32, isOutput=True)

    chunks = list(cfg.p_chunks())
    chunk_at = {(g, b): (row0, n_g, bd, ofs, size)
                for g, b, row0, n_g, bd, ofs, size in chunks}

    with tile.TileContext(nc) as tc:
        with (
            tc.tile_pool(name="const", bufs=1) as cpool,
            tc.tile_pool(name="E", bufs=cfg.E_BUFS) as epool,
            tc.tile_pool(name="stage", bufs=cfg.STAGE_BUFS) as spool,
            tc.tile_pool(name="stage_bf", bufs=cfg.STAGE_BUFS) as sbfpool,
            tc.tile_pool(name="rsmall", bufs=2) as rpool,
            tc.tile_pool(name="fence", bufs=max(2, cfg.G)) as fpool,
            tc.tile_pool(name="S", bufs=cfg.S_BUFS, space="PSUM") as s_psum,
            tc.tile_pool(name="ACC", bufs=2, space="PSUM") as acc_psum,
            tc.tile_pool(name="RB", bufs=1, space="PSUM") as r_psum,
        ):
            # resident inputs
            qT_sb = cpool.tile([64, cfg.Ncore], bf16)
            kT_sb = cpool.tile([64, M], bf16)
            vE_sb = cpool.tile([128, MT * 65], bf16)
            ones_sb = cpool.tile([1, 128], fp32)
            resT_sb = cpool.tile([D, cfg.Ncore], fp32)
            if cfg.SPLIT_LOADS:
                # Split the resident-input loads into chunks so group 0's
                # first matmuls start as soon as their k/q/v slices land
                # instead of waiting for whole-tensor DMAs.
                n0 = cfg.GROUPS[0]
                nc.sync.dma_start(qT_sb[:, 0:n0], qT[:, 0:n0])
                for c4 in range(4):
                    sl = ds(c4 * M // 4, M // 4)
                    nc.sync.dma_start(kT_sb[:, sl], kT[:, sl])
                    sl2 = ds(c4 * (MT // 4) * 65, (MT // 4) * 65)
                    nc.sync.dma_start(vE_sb[:, sl2], vE[:, sl2])
                if n0 < cfg.Ncore:
                    nc.sync.dma_start(qT_sb[:, n0:], qT[:, n0:])
            else:
                nc.sync.dma_start(qT_sb[:], qT[:])
                nc.sync.dma_start(kT_sb[:], kT[:])
                nc.sync.dma_start(vE_sb[:], vE[:])
            # DVE memset (not gpsimd): the R-broadcast matmul depends on both
            # this and the DVE reciprocal; PE instructions fit only ONE sync
            # wait, so both deps must land on the same (DVE) semaphore.
            nc.vector.memset(ones_sb[:], 1.0)
            # Dummy exp up front: forces the ACT spline-table load to happen
            # during the input-DMA window.  On a freshly loaded NEFF the first
            # real exp otherwise races the table load and yields garbage.
            scratch_act = cpool.tile([1, 2], fp32)
            nc.scalar.activation(scratch_act[:], ones_sb[0:1, 0:2],
                                 mybir.ActivationFunctionType.Exp)

            row0 = 0
            dma_idx = 0
            for g, n in enumerate(cfg.GROUPS):
                b_exp = cfg.B_EXP_G[g]
                E = epool.tile([128, MT * n], bf16, tag="E")  # exp(S_T)
                acc = acc_psum.tile([65, n], fp32, tag="acc")  # [V|1]^T @ E
                for blk in range(MT // b_exp):
                    S = s_psum.tile([128, b_exp * n], fp32, tag="S")
                    for i in range(b_exp):
                        t = blk * b_exp + i
                        nc.tensor.matmul(
                            S[:, ts(i, n)], kT_sb[:, ts(t, 128)],
                            qT_sb[:, ds(row0, n)],
                            start=True, stop=True,
                        )
                    nc.scalar.activation(
                        E[:, ds(blk * b_exp * n, b_exp * n)], S[:],
                        mybir.ActivationFunctionType.Exp,
                    )
                    for i in range(b_exp):
                        t = blk * b_exp + i
                        nc.tensor.matmul(
                            acc[:], vE_sb[:, ts(t, 65)], E[:, ts(t, n)],
                            start=(t == 0), stop=(t == MT - 1),
                        )
                # softmax denominators for this group: acc row 64 holds sums
                r = rpool.tile([1, n], fp32, tag="r")
                nc.vector.reciprocal(r[:], acc[64:65, :])
                Rps = r_psum.tile([128, n], fp32, tag="Rps")
                nc.tensor.matmul(Rps[:], ones_sb[:], r[:], start=True, stop=True)
                Rbf = rpool.tile([128, n], bf16, tag="Rbf")
                nc.vector.tensor_copy(Rbf[:], Rps[:])
                Rf32 = rpool.tile([128, n], fp32, tag="Rf32")
                nc.vector.tensor_copy(Rf32[:], Rps[:])
                # res_T slice for this group = acc[0:64] * r (broadcast over d)
                nc.vector.tensor_tensor(
                    resT_sb[:, ds(row0, n)], acc[0:D, :], Rf32[0:D, :],
                    mybir.AluOpType.mult,
                )
                # ACT fence on DVE: one op that waits for the group's last exp,
                # so the phase-C TTs' E-read deps are elided and each TT keeps
                # a single sync wait (its stage-slot WAR vs the output DMA) —
                # ISA instructions fit only one wait.
                fence = fpool.tile([1, 1], bf16, tag="fence")
                nc.vector.tensor_copy(fence[:], E[0:1, MT * n - 1:MT * n])
                # normalize + stream p tiles out.  Alternate DMA issue between
                # SP-HWDGE and Pool-SWDGE — independent descriptor queues, so
                # the output stream pipelines across both.  SWDGE batches use
                # a bf16 stage (DVE runs the multiply at 2x for 2-byte in/out)
                # and cast bf16->f32 during the DMA; HWDGE can't cast, so its
                # batches stage in f32.
                b_dma = cfg.B_DMA_G[g]
                for b in range(MT // b_dma):
                    _, n_g, bd, ofs, size = chunk_at[(g, b)]
                    use_swdge = (dma_idx % 2 == 1) and "gpsimd" in cfg.DMA_ENGS
                    dma_idx += 1
                    if use_swdge and cfg.BF16_STAGE:
                        dst = sbfpool.tile([128, bd * n], bf16, tag="stage_bf")
                        eng = nc.gpsimd
                    else:
                        dst = spool.tile([128, bd * n], fp32, tag="stage")
                        eng = nc.gpsimd if use_swdge else nc.sync
                    nc.vector.tensor_tensor(
                        dst[:].rearrange("p (i c) -> p i c", i=bd),
                        E[:, ds(b * bd * n, bd * n)].rearrange(
                            "p (i c) -> p i c", i=bd),
                        Rbf[:, None, :].to_broadcast([128, bd, n]),
                        mybir.AluOpType.mult,
                    )
                    eng.dma_start(
                        p_out[ofs:ofs + size].rearrange("(p x) -> p x", p=128),
                        dst[:],
                    )
                row0 += n
            # res goes out on the (otherwise idle) ACT HWDGE queue so it does
            # not delay the last p batches on the SP queue.
            nc.scalar.dma_start(resT_out[:], resT_sb[:])
    _split_multi_wait_instructions(nc)
    return nc


def prep_inputs(query, key, value, hyper_c, cfg: Cfg, n_cores=N_CORES):
    """Host-side layout prep: normalize rows, fold hyper_c, bf16, transpose."""
    q = np.asarray(query, dtype=np.float32)
    k = np.asarray(key, dtype=np.float32)
    v = np.asarray(value, dtype=np.float32)
    hc = float(hyper_c)
    MT, D = cfg.MT, cfg.D

    qn_norm = np.linalg.norm(q, axis=1, keepdims=True)
    kn_norm = np.linalg.norm(k, axis=1, keepdims=True)
    qn = (q * (hc / np.where(qn_norm == 0, 1.0, qn_norm))).astype(BF16)
    kn = (k / np.where(kn_norm == 0, 1.0, kn_norm)).astype(BF16)

    # kT: [64, M] = normalized keys transposed (d on partitions)
    kT_h = np.ascontiguousarray(kn.T)
    # vE: [128, MT*65]; row p, col t*65+c -> v[t*128+p, c] for c<64, 1.0 at c=64
    vE = np.ones((MT, 128, 65), dtype=np.float32)
    vE[:, :, :D] = v.reshape(MT, 128, D)
    vE_h = np.ascontiguousarray(vE.astype(BF16).transpose(1, 0, 2).reshape(128, MT * 65))

    qT_all = qn.T  # [64, N]
    in_maps = []
    for c in range(n_cores):
        qT_h = np.ascontiguousarray(qT_all[:, c * cfg.Ncore:(c + 1) * cfg.Ncore])
        in_maps.append({"qT": qT_h, "kT": kT_h, "vE": vE_h})
    return in_maps


def gather_outputs(results, cfg: Cfg, n_cores=N_CORES):
    N = cfg.Ncore * n_cores
    p_attn = np.empty((N, cfg.M), dtype=np.float32)
    res = np.empty((N, cfg.D), dtype=np.float32)
    chunks = list(cfg.p_chunks())
    for c in range(n_cores):
        po = results[c]["p_out"]          # flat [Ncore*M]
        rt = results[c]["resT_out"]       # [D, Ncore]
        base = c * cfg.Ncore
        for g, b, row0, n_g, bd, ofs, size in chunks:
            chunk = po[ofs:ofs + size].reshape(128, bd, n_g)
            cols = b * bd * 128
            # chunk[p, i, c2] -> p_attn[base+row0+c2, cols + i*128 + p]
            p_attn[base + row0:base + row0 + n_g,
                   cols:cols + bd * 128] = (
                chunk.transpose(2, 1, 0).reshape(n_g, bd * 128))
        res[base:base + cfg.Ncore, :] = rt.T
    return res, p_attn


_NC_CACHE = {}


def _get_nc(cfg_key=None):
    if cfg_key is None:
        cfg = Cfg()
        key = "full"
    else:
        cfg, key = cfg_key
    if key not in _NC_CACHE:
        _NC_CACHE[key] = (build_nc(cfg), cfg)
    return _NC_CACHE[key]


def run(inputs, trace=False, cfg=None, warmup=True, **spmd_kwargs):
    """Full pipeline; returns ((res, p_attn), BassKernelResults).

    warmup: the FIRST execution of a freshly loaded NEFF can race the cold
    DMA/engine pipelines (observed: corrupted tails of the first score
    matmuls on cold runs only; every warm execution is deterministic and
    correct).  Run once to warm the pipelines, then return the second run.
    """
    if cfg is None:
        nc, cfg = _get_nc()
    else:
        nc = build_nc(cfg)
    in_maps = prep_inputs(inputs["query"], inputs["key"], inputs["value"],
                          inputs.get("hyper_c", 1), cfg)
    core_ids = list(range(N_CORES))
    if warmup:
        run_bass_kernel_spmd(nc, in_maps, core_ids=core_ids, trace=False)
    out = run_bass_kernel_spmd(nc, in_maps, core_ids=core_ids,
                               trace=trace, **spmd_kwargs)
    return gather_outputs(out.results, cfg), out


def kernel(**inputs):
    (res, p_attn), _ = run(inputs, trace=False)
    return res, p_attn
